# revision 24
# baseline (speedup 1.0000x reference)
"""Trainium2 Bass kernel for nn_MultiHeadAttention (B=4, S=2048, D=1024, H=16).

Sharding over 8 NeuronCores: core c -> (batch b = c//2, head-group g = c%2).
Each core computes, for its batch and its 8 heads:
  - projections in transposed layout: qhT/khT [512, 2048] = relu(W x^T + b),
    vh in normal layout [2048, 512]
  - per head: scores twice (q-major for softmax/output write, k-major for
    attn@V), exp on ScalarE with fused row-sum (softmax denominator),
    normalization on VectorE, unnormalized attn@V on PE with the
    normalization folded into the PSUM eviction
  - output projection row-split across the 2 cores of a batch after a
    pairwise AllToAll exchange of y^T halves.
Outputs per core: its [2048, 16384] slab of attn_weights and its
[1024, 1024] slab of y.
"""

import os
import sys

for _p in ("/opt/trn_rl_repo", "/opt/pypackages"):
    if _p not in sys.path and os.path.isdir(_p):
        sys.path.append(_p)

import numpy as np

import concourse.bass as bass
import concourse.tile as tile
from concourse import bacc
from concourse import mybir
from concourse.bass_utils import run_bass_kernel_spmd

F32 = mybir.dt.float32
AF = mybir.ActivationFunctionType
ALU = mybir.AluOpType

P = 128          # partitions
B = 4            # batch
S = 2048         # sequence length
D = 1024         # model dim
H = 16           # total heads
DH = 64          # head dim
HL = 8           # heads per core (head-group size)
DG = HL * DH     # 512, projected dims per core
NPAIR = HL // 2  # 4 local head pairs
KT = D // P      # 8 contraction tiles for projections
QM = S // P      # 16 q-row tiles
SCALE = 1.0 / 8.0  # 1/sqrt(DH)

USE_COLLECTIVE = True
MM_DT = os.environ.get("MM_DT", "f32r")  # f32r | f32


_CACHE = {}


MDT = mybir.dt.float32r if os.environ.get("MM_DT", "f32r") == "f32r" else mybir.dt.float32


def _mm_ap(ap):
    return ap


def _build_nc():
    nc = bacc.Bacc("TRN2", target_bir_lowering=False, debug=False, num_devices=8)

    # ---- dram I/O ----
    qT = nc.dram_tensor("qT", [D, S], MDT, kind="ExternalInput")
    kT = nc.dram_tensor("kT", [D, S], MDT, kind="ExternalInput")
    vT = nc.dram_tensor("vT", [D, S], MDT, kind="ExternalInput")
    wq = nc.dram_tensor("wq", [D, DG], MDT, kind="ExternalInput")
    wk = nc.dram_tensor("wk", [D, DG], MDT, kind="ExternalInput")
    wv = nc.dram_tensor("wv", [D, DG], MDT, kind="ExternalInput")
    wo = nc.dram_tensor("wo", [D, D], MDT, kind="ExternalInput")
    bq = nc.dram_tensor("bq", [P, NPAIR], F32, kind="ExternalInput")
    bk = nc.dram_tensor("bk", [P, NPAIR], F32, kind="ExternalInput")
    bv = nc.dram_tensor("bv", [1, DG], F32, kind="ExternalInput")
    bo = nc.dram_tensor("bo", [1, D], F32, kind="ExternalInput")
    zeros_in = nc.dram_tensor("zeros_in", [1, QM * NPAIR * 256], MDT, kind="ExternalInput")

    attn_out = nc.dram_tensor("attn_out", [S, HL * S], F32, kind="ExternalOutput")
    y_out = nc.dram_tensor("y_out", [S, D], F32, kind="ExternalOutput")

    # per-head softmax denominator bounce (for partition->free transposition)
    recip_dram = nc.dram_tensor("recip_dram", [HL, S], F32)
    if USE_COLLECTIVE:
        # pairwise AllGather: both cores of a batch receive [yT_g0; yT_g1],
        # split into 4 per-pair chunks so the exchange overlaps attention
        yt_bounce = nc.dram_tensor("yt_bounce", [DG, S], MDT)
        yt_allp = [nc.dram_tensor(f"yt_all{p}", [2, P, S], MDT) for p in range(NPAIR)]

    with tile.TileContext(nc) as tc:
        _emit(nc, tc, locals())
    if not nc.is_finalized():
        nc.finalize()
    return nc


def _emit_ag(nc, t, pr):
    yt_bounce = t["yt_bounce"]
    ag = nc.gpsimd.collective_compute(
        "AllGather",
        ALU.bypass,
        replica_groups=[[0, 1], [2, 3], [4, 5], [6, 7]],
        ins=[yt_bounce.ap()[pr * P:(pr + 1) * P, :].opt()],
        outs=[t["yt_allp"][pr].ap().opt()],
    )
    return ag


def _emit(nc, tc, t):
    from contextlib import ExitStack

    qT, kT, vT = t["qT"], t["kT"], t["vT"]
    wq, wk, wv, wo = t["wq"], t["wk"], t["wv"], t["wo"]
    bq, bk, bv, bo = t["bq"], t["bk"], t["bv"], t["bo"]
    attn_out, y_out, recip_dram = t["attn_out"], t["y_out"], t["recip_dram"]

    with ExitStack() as ctx:
        # ---------- constants ----------
        const = ctx.enter_context(tc.tile_pool(name="const", bufs=1))
        bq_sb = const.tile([P, NPAIR], F32, tag="bq")
        bk_sb = const.tile([P, NPAIR], F32, tag="bk")
        bv_sb = const.tile([P, DG], F32, tag="bv")
        bo_sb = const.tile([P, D], F32, tag="bo")
        nc.sync.dma_start(out=bq_sb, in_=bq.ap())
        nc.sync.dma_start(out=bk_sb, in_=bk.ap())
        nc.sync.dma_start(out=bv_sb, in_=bass.AP(tensor=bv, offset=0, ap=[[0, P], [1, DG]]))
        nc.sync.dma_start(out=bo_sb, in_=bass.AP(tensor=bo, offset=0, ap=[[0, P], [1, D]]))

        # ---------- resident activations ----------
        # qhT/khT: [128, pair-block p * 2048 + s]; partitions 0:64 = head 2p,
        # 64:128 = head 2p+1 (feature dim on partitions).
        # vh: [128, s-tile st * 512 + d_local] (sequence on partitions).
        # qhT doubles as the y^T staging buffer late in the kernel.
        qy_pool = ctx.enter_context(tc.tile_pool(name="qy", bufs=1))
        qhT = qy_pool.tile([P, NPAIR * S], MDT, tag="qhT")

        with ExitStack() as attn_ctx:
            kv_pool = attn_ctx.enter_context(tc.tile_pool(name="kv", bufs=1))
            khT = kv_pool.tile([P, NPAIR * S], MDT, tag="khT")
            vh = kv_pool.tile([P, QM * NPAIR * 256], MDT, tag="vh")
            zsrc = bass.AP(tensor=t["zeros_in"], offset=0,
                           ap=[[0, P], [1, QM * NPAIR * 256]])
            nc.sync.dma_start(out=vh, in_=zsrc)

            # ---------- phase A: projections ----------
            with ExitStack() as proj_ctx:
                wpool = proj_ctx.enter_context(tc.tile_pool(name="wts", bufs=1))
                wq_sb = wpool.tile([P, KT, DG], MDT, tag="wq")
                wk_sb = wpool.tile([P, KT, DG], MDT, tag="wk")
                wv_sb = wpool.tile([P, KT, DG], MDT, tag="wv")
                for w_dram, w_sb in ((wq, wq_sb), (wk, wk_sb), (wv, wv_sb)):
                    for kt in range(KT):
                        nc.sync.dma_start(
                            out=w_sb[:, kt, :],
                            in_=w_dram.ap()[kt * P:(kt + 1) * P, :],
                        )

                xpool = proj_ctx.enter_context(tc.tile_pool(name="xs", bufs=1))

                # q/k projections (transposed form, bias per partition)
                with tc.tile_pool(name="pjqk", bufs=1, space="PSUM") as ppool:
                    for x_dram, w_sb, b_sb, dst in (
                        (qT, wq_sb, bq_sb, qhT),
                        (kT, wk_sb, bk_sb, khT),
                    ):
                        for nch2 in range(2):
                            psums = [ppool.tile([P, 1024], F32, tag=f"pj{p}", name=f"pj{p}", bufs=1)
                                     for p in range(NPAIR)]
                            for kt in range(KT):
                                xt = xpool.tile([P, 1024], MDT, tag="xt", bufs=4)
                                nc.sync.dma_start(
                                    out=xt,
                                    in_=x_dram.ap()[kt * P:(kt + 1) * P,
                                                    nch2 * 1024:(nch2 + 1) * 1024],
                                )
                                for p in range(NPAIR):
                                    for hf in range(2):
                                        nc.tensor.matmul(
                                            psums[p][:, hf * 512:(hf + 1) * 512],
                                            lhsT=_mm_ap(w_sb[:, kt, p * P:(p + 1) * P]),
                                            rhs=_mm_ap(xt[:, hf * 512:(hf + 1) * 512]),
                                            start=(kt == 0),
                                            stop=(kt == KT - 1),
                                        )
                            for p in range(NPAIR):
                                nc.scalar.activation(
                                    dst[:, p * S + nch2 * 1024: p * S + (nch2 + 1) * 1024],
                                    psums[p],
                                    AF.Relu,
                                    bias=b_sb[:, p: p + 1],
                                )

                # v projection (normal form, bias along free dim via DVE)
                with tc.tile_pool(name="pjv", bufs=1, space="PSUM") as vpool:
                    for stg in range(4):
                        pvs = [vpool.tile([P, 512], F32, tag=f"pv{i}", name=f"pv{i}", bufs=2)
                               for i in range(4)]
                        for kt in range(KT):
                            vt = xpool.tile([P, 512], MDT, tag="vt", bufs=4)
                            nc.sync.dma_start(
                                out=vt,
                                in_=vT.ap()[kt * P:(kt + 1) * P, stg * 512:(stg + 1) * 512],
                            )
                            for i in range(4):
                                nc.tensor.matmul(
                                    pvs[i],
                                    lhsT=_mm_ap(vt[:, i * P:(i + 1) * P]),
                                    rhs=_mm_ap(wv_sb[:, kt, :]),
                                    start=(kt == 0),
                                    stop=(kt == KT - 1),
                                )
                        for i in range(4):
                            st = stg * 4 + i
                            vslc = bass.AP(
                                tensor=vh.tensor,
                                offset=vh.offset + st * 1024,
                                ap=[list(vh.ap[0]), [256, NPAIR], [192, 2], [1, DH]],
                            )
                            nc.vector.tensor_tensor(vslc, pvs[i], bv_sb, ALU.add)
                            nc.vector.tensor_scalar(vslc, vslc, 0.0, None, ALU.max)

            # ---------- phase B: attention ----------
            with ExitStack() as b_ctx:
                apool = b_ctx.enter_context(tc.tile_pool(name="attn", bufs=3))
                epool = b_ctx.enter_context(tc.tile_pool(name="expt", bufs=3))
                rpool = b_ctx.enter_context(tc.tile_pool(name="recip", bufs=2))
                bpool = b_ctx.enter_context(tc.tile_pool(name="rbcast", bufs=2))
                ypool = b_ctx.enter_context(tc.tile_pool(name="ystage", bufs=2))
                sq_ps = b_ctx.enter_context(tc.tile_pool(name="sq", bufs=1, space="PSUM"))
                st_ps = b_ctx.enter_context(tc.tile_pool(name="st", bufs=1, space="PSUM"))
                yt_ps = b_ctx.enter_context(tc.tile_pool(name="yt", bufs=1, space="PSUM"))
                yt_bounce = t.get("yt_bounce")

                def emit_A(pr, hh, m, rec):
                    """q-major scores + softmax + attn output write for one q tile."""
                    h = 2 * pr + hh
                    qs = slice(64 * hh, 64 * hh + 64)
                    ps = sq_ps.tile([P, S], F32, tag="sq", name="sq")
                    for nch in range(4):
                        nc.tensor.matmul(
                            ps[:, nch * 512:(nch + 1) * 512],
                            lhsT=_mm_ap(qhT[qs, pr * S + m * P: pr * S + (m + 1) * P]),
                            rhs=_mm_ap(khT[qs, pr * S + nch * 512: pr * S + (nch + 1) * 512]),
                            start=True,
                            stop=True,
                        )
                    at = apool.tile([P, S], F32, tag="at", name="at")
                    den = rec[:, m: m + 1]
                    nc.scalar.activation(at, ps, AF.Exp, scale=SCALE, accum_out=den)
                    nc.vector.reciprocal(den, den)
                    nc.vector.tensor_scalar(at, at, den, None, ALU.mult)
                    nc.sync.dma_start(
                        out=attn_out.ap()[m * P:(m + 1) * P, h * S:(h + 1) * S],
                        in_=at,
                    )

                def emit_rb(h, rec):
                    """denominators: partition-form -> free-form broadcast tile."""
                    nc.gpsimd.dma_start(
                        out=recip_dram.ap()[h, :].rearrange("(m u) -> u m", u=P),
                        in_=rec,
                    )
                    rb = bpool.tile([P, S], F32, tag="rb", name="rb")
                    nc.gpsimd.dma_start(
                        out=rb,
                        in_=bass.AP(tensor=recip_dram, offset=h * S, ap=[[0, P], [1, S]]),
                    )
                    return rb

                def gen_B(pr, rbs):
                    """k-major scores + exp + attn@V; yields after each kt unit."""
                    for qh2 in range(2):
                        pyt = yt_ps.tile([P, S // 2], F32, tag="pyt", name="pyt")
                        for hh in range(2):
                            qs = slice(64 * hh, 64 * hh + 64)
                            for kt in range(QM):
                                pst = st_ps.tile([P, S // 2], F32, tag="pst", name="pst")
                                for nc2 in range(2):
                                    nc.tensor.matmul(
                                        pst[:, nc2 * 512:(nc2 + 1) * 512],
                                        lhsT=_mm_ap(khT[qs, pr * S + kt * P: pr * S + (kt + 1) * P]),
                                        rhs=_mm_ap(qhT[qs, pr * S + qh2 * 1024 + nc2 * 512:
                                                pr * S + qh2 * 1024 + (nc2 + 1) * 512]),
                                        start=True,
                                        stop=True,
                                    )
                                et = epool.tile([P, S // 2], MDT, tag="et", name="et")
                                nc.scalar.activation(et, pst, AF.Exp, scale=SCALE)
                                for nc2 in range(2):
                                    nc.tensor.matmul(
                                        pyt[:, nc2 * 512:(nc2 + 1) * 512],
                                        lhsT=_mm_ap(vh[:, kt * 1024 + pr * 256 + hh * P:
                                                       kt * 1024 + pr * 256 + (hh + 1) * P]),
                                        rhs=_mm_ap(et[:, nc2 * 512:(nc2 + 1) * 512]),
                                        start=(kt == 0 and hh == 0),
                                        stop=(kt == QM - 1 and hh == 1),
                                    )
                                yield
                        # evict yT pair tile to DRAM, folding in normalization
                        ys = ypool.tile([P, S // 2], MDT, tag="ys", name="ys")
                        for hh in range(2):
                            qs = slice(64 * hh, 64 * hh + 64)
                            nc.vector.tensor_tensor(
                                ys[qs, :], pyt[qs, :],
                                rbs[hh][qs, qh2 * 1024:(qh2 + 1) * 1024],
                                ALU.mult,
                            )
                        if yt_bounce is not None:
                            nc.sync.dma_start(
                                out=yt_bounce.ap()[pr * P:(pr + 1) * P,
                                                   qh2 * 1024:(qh2 + 1) * 1024],
                                in_=ys,
                            )
                        yield

                ag_insts = []
                for pr in range(NPAIR):
                    # denominator tiles for both heads of the pair
                    recs = [rpool.tile([P, QM], F32, tag=f"rec{hh}", name=f"rec{hh}")
                            for hh in range(2)]
                    rbs = [None, None]

                    bgen = gen_B(pr - 1, _prev_rbs) if pr > 0 else iter(())
                    bdone = pr == 0
                    for hh in range(2):
                        for m in range(QM):
                            emit_A(pr, hh, m, recs[hh])
                            for _ in range(2):
                                if not bdone:
                                    try:
                                        next(bgen)
                                    except StopIteration:
                                        bdone = True
                        rbs[hh] = emit_rb(2 * pr + hh, recs[hh])
                    if not bdone:
                        for _ in bgen:
                            pass
                    if pr > 0 and USE_COLLECTIVE:
                        ag_insts.append(_emit_ag(nc, t, pr - 1))
                    _prev_rbs = rbs

                # drain the last pair's B stream
                for _ in gen_B(NPAIR - 1, _prev_rbs):
                    pass
                if USE_COLLECTIVE:
                    ag_insts.append(_emit_ag(nc, t, NPAIR - 1))

        # ---------- phase C: output projection ----------
        with ExitStack() as c_ctx:
            if USE_COLLECTIVE:
                opool = c_ctx.enter_context(tc.tile_pool(name="oproj", bufs=1))
                ytf = opool.tile([P, KT, S], MDT, tag="ytf")
                wo_sb = opool.tile([P, KT, D], MDT, tag="wo")
                # kk = j*4 + p (global dcol tile); load order p-major so the
                # last pair's chunks arrive last
                kk_order = [jj * NPAIR + pp for pp in range(NPAIR) for jj in range(2)]
                for kk in kk_order:
                    jj, pp = kk // NPAIR, kk % NPAIR
                    d2 = nc.sync.dma_start(
                        out=ytf[:, kk, :],
                        in_=t["yt_allp"][pp].ap()[jj, :, :],
                    )
                    tile.add_dep_helper(d2.ins, ag_insts[pp].ins, True, "ytf after AG")
                for kk in range(KT):
                    nc.sync.dma_start(
                        out=wo_sb[:, kk, :], in_=wo.ap()[kk * P:(kk + 1) * P, :]
                    )
                ypool = c_ctx.enter_context(tc.tile_pool(name="yev", bufs=2))
                o_ps = c_ctx.enter_context(tc.tile_pool(name="ops", bufs=4, space="PSUM"))
                for m in range(S // P):
                    po = o_ps.tile([P, D], F32, tag="po")
                    for ki, kk in enumerate(kk_order):
                        for nc2 in range(2):
                            nc.tensor.matmul(
                                po[:, nc2 * 512:(nc2 + 1) * 512],
                                lhsT=_mm_ap(ytf[:, kk, m * P:(m + 1) * P]),
                                rhs=_mm_ap(wo_sb[:, kk, nc2 * 512:(nc2 + 1) * 512]),
                                start=(ki == 0),
                                stop=(ki == KT - 1),
                            )
                    yo = ypool.tile([P, D], F32, tag="yo")
                    nc.vector.tensor_tensor(yo, po, bo_sb, ALU.add)
                    nc.vector.tensor_scalar(yo, yo, 0.0, None, ALU.max)
                    nc.sync.dma_start(out=y_out.ap()[m * P:(m + 1) * P, :], in_=yo)


def _get_nc():
    if "nc" not in _CACHE:
        _CACHE["nc"] = _build_nc()
    return _CACHE["nc"]


def _in_maps(q, k, v, Wq, bq, Wk, bk, Wv, bv, Wo, bo):
    maps = []
    WoT = np.ascontiguousarray(Wo.T)
    for c in range(8):
        b, g = c // 2, c % 2
        gs = slice(g * DG, (g + 1) * DG)
        maps.append({
            "qT": np.ascontiguousarray(q[b].T),
            "kT": np.ascontiguousarray(k[b].T),
            "vT": np.ascontiguousarray(v[b].T),
            "wq": np.ascontiguousarray(Wq[gs, :].T),
            "wk": np.ascontiguousarray(Wk[gs, :].T),
            "wv": np.ascontiguousarray(Wv[gs, :].T),
            "wo": WoT,
            "bq": np.ascontiguousarray(bq[gs].reshape(NPAIR, P).T),
            "bk": np.ascontiguousarray(bk[gs].reshape(NPAIR, P).T),
            "bv": np.ascontiguousarray(bv[gs].reshape(1, DG)),
            "bo": np.ascontiguousarray(bo.reshape(1, D)),
            "zeros_in": np.zeros((1, QM * NPAIR * 256), np.float32),
        })
    return maps


def run_spmd(inputs, **kwargs):
    """Run the SPMD kernel; returns (results_per_core, BassKernelResults)."""
    nc = _get_nc()
    in_maps = _in_maps(**inputs)
    res = run_bass_kernel_spmd(nc, in_maps, core_ids=list(range(8)), **kwargs)
    return res.results, res


def assemble(results):
    y = np.empty((B, S, D), dtype=np.float32)
    attn = np.empty((B, S, H * S), dtype=np.float32)
    for c in range(8):
        b, g = c // 2, c % 2
        attn[b, :, g * HL * S:(g + 1) * HL * S] = results[c]["attn_out"]
        half = slice(g * (S // 2), (g + 1) * (S // 2))
        y[b, half, :] = results[c]["y_out"][half, :]
    return y, attn


def kernel(**inputs):
    results, _ = run_spmd(inputs)
    return assemble(results)


# revision 26
# speedup vs baseline: 1.0320x; 1.0320x over previous
"""Trainium2 Bass kernel for nn_MultiHeadAttention (B=4, S=2048, D=1024, H=16).

Sharding over 8 NeuronCores: core c -> (batch b = c//2, head-group g = c%2).
Each core computes, for its batch and its 8 heads:
  - projections in transposed layout: qhT/khT [512, 2048] = relu(W x^T + b),
    vh in normal layout [2048, 512]
  - per head: scores twice (q-major for softmax/output write, k-major for
    attn@V), exp on ScalarE with fused row-sum (softmax denominator),
    normalization on VectorE, unnormalized attn@V on PE with the
    normalization folded into the PSUM eviction
  - output projection row-split across the 2 cores of a batch after a
    pairwise AllToAll exchange of y^T halves.
Outputs per core: its [2048, 16384] slab of attn_weights and its
[1024, 1024] slab of y.
"""

import os
import sys

for _p in ("/opt/trn_rl_repo", "/opt/pypackages"):
    if _p not in sys.path and os.path.isdir(_p):
        sys.path.append(_p)

import numpy as np

import concourse.bass as bass
import concourse.tile as tile
from concourse import bacc
from concourse import mybir
from concourse.bass_utils import run_bass_kernel_spmd

F32 = mybir.dt.float32
AF = mybir.ActivationFunctionType
ALU = mybir.AluOpType

P = 128          # partitions
B = 4            # batch
S = 2048         # sequence length
D = 1024         # model dim
H = 16           # total heads
DH = 64          # head dim
HL = 8           # heads per core (head-group size)
DG = HL * DH     # 512, projected dims per core
NPAIR = HL // 2  # 4 local head pairs
KT = D // P      # 8 contraction tiles for projections
QM = S // P      # 16 q-row tiles
SCALE = 1.0 / 8.0  # 1/sqrt(DH)

USE_COLLECTIVE = True
MM_DT = os.environ.get("MM_DT", "f32r")  # f32r | f32


_CACHE = {}


MDT = mybir.dt.float32r if os.environ.get("MM_DT", "f32r") == "f32r" else mybir.dt.float32


def _mm_ap(ap):
    return ap


def _build_nc():
    nc = bacc.Bacc("TRN2", target_bir_lowering=False, debug=False, num_devices=8)

    # ---- dram I/O ----
    qT = nc.dram_tensor("qT", [D, S], MDT, kind="ExternalInput")
    kT = nc.dram_tensor("kT", [D, S], MDT, kind="ExternalInput")
    vT = nc.dram_tensor("vT", [D, S], MDT, kind="ExternalInput")
    wq = nc.dram_tensor("wq", [D, DG], MDT, kind="ExternalInput")
    wk = nc.dram_tensor("wk", [D, DG], MDT, kind="ExternalInput")
    wv = nc.dram_tensor("wv", [D, DG], MDT, kind="ExternalInput")
    wo = nc.dram_tensor("wo", [D, D], MDT, kind="ExternalInput")
    bq = nc.dram_tensor("bq", [P, NPAIR], F32, kind="ExternalInput")
    bk = nc.dram_tensor("bk", [P, NPAIR], F32, kind="ExternalInput")
    bv = nc.dram_tensor("bv", [1, DG], F32, kind="ExternalInput")
    bo = nc.dram_tensor("bo", [1, D], F32, kind="ExternalInput")

    attn_out = nc.dram_tensor("attn_out", [S, HL * S], F32, kind="ExternalOutput")
    y_out = nc.dram_tensor("y_out", [S, D], F32, kind="ExternalOutput")

    # per-head softmax denominator bounce (for partition->free transposition)
    recip_dram = nc.dram_tensor("recip_dram", [HL, S], F32)
    if USE_COLLECTIVE:
        # pairwise AllGather: both cores of a batch receive [yT_g0; yT_g1],
        # split into 4 per-pair chunks so the exchange overlaps attention
        yt_bounce = nc.dram_tensor("yt_bounce", [DG, S], MDT)
        yt_allp = [nc.dram_tensor(f"yt_all{p}", [2, P, S], MDT) for p in range(NPAIR)]

    with tile.TileContext(nc) as tc:
        _emit(nc, tc, locals())
    if not nc.is_finalized():
        nc.finalize()
    return nc


def _emit_ag(nc, t, pr):
    yt_bounce = t["yt_bounce"]
    ag = nc.gpsimd.collective_compute(
        "AllGather",
        ALU.bypass,
        replica_groups=[[0, 1], [2, 3], [4, 5], [6, 7]],
        ins=[yt_bounce.ap()[pr * P:(pr + 1) * P, :].opt()],
        outs=[t["yt_allp"][pr].ap().opt()],
    )
    return ag


def _emit(nc, tc, t):
    from contextlib import ExitStack

    qT, kT, vT = t["qT"], t["kT"], t["vT"]
    wq, wk, wv, wo = t["wq"], t["wk"], t["wv"], t["wo"]
    bq, bk, bv, bo = t["bq"], t["bk"], t["bv"], t["bo"]
    attn_out, y_out, recip_dram = t["attn_out"], t["y_out"], t["recip_dram"]

    with ExitStack() as ctx:
        # ---------- constants ----------
        const = ctx.enter_context(tc.tile_pool(name="const", bufs=1))
        bq_sb = const.tile([P, NPAIR], F32, tag="bq")
        bk_sb = const.tile([P, NPAIR], F32, tag="bk")
        bv_sb = const.tile([P, DG], F32, tag="bv")
        bo_sb = const.tile([P, D], F32, tag="bo")
        nc.sync.dma_start(out=bq_sb, in_=bq.ap())
        nc.sync.dma_start(out=bk_sb, in_=bk.ap())
        nc.sync.dma_start(out=bv_sb, in_=bass.AP(tensor=bv, offset=0, ap=[[0, P], [1, DG]]))
        nc.sync.dma_start(out=bo_sb, in_=bass.AP(tensor=bo, offset=0, ap=[[0, P], [1, D]]))

        # ---------- resident activations ----------
        # qhT/khT: [128, pair-block p * 2048 + s]; partitions 0:64 = head 2p,
        # 64:128 = head 2p+1 (feature dim on partitions).
        # vh: [128, s-tile st * 512 + d_local] (sequence on partitions).
        # qhT doubles as the y^T staging buffer late in the kernel.
        qy_pool = ctx.enter_context(tc.tile_pool(name="qy", bufs=1))
        qhT = qy_pool.tile([P, NPAIR * S], MDT, tag="qhT")

        with ExitStack() as attn_ctx:
            kv_pool = attn_ctx.enter_context(tc.tile_pool(name="kv", bufs=1))
            khT = kv_pool.tile([P, NPAIR * S], MDT, tag="khT")
            vh = kv_pool.tile([P, QM * DG], mybir.dt.bfloat16, tag="vh")

            # ---------- phase A: projections ----------
            with ExitStack() as proj_ctx:
                wpool = proj_ctx.enter_context(tc.tile_pool(name="wts", bufs=1))
                wq_sb = wpool.tile([P, KT, DG], MDT, tag="wq")
                wk_sb = wpool.tile([P, KT, DG], MDT, tag="wk")
                wv_sb = wpool.tile([P, KT, DG], MDT, tag="wv")
                for w_dram, w_sb in ((wq, wq_sb), (wk, wk_sb), (wv, wv_sb)):
                    for kt in range(KT):
                        nc.sync.dma_start(
                            out=w_sb[:, kt, :],
                            in_=w_dram.ap()[kt * P:(kt + 1) * P, :],
                        )

                xpool = proj_ctx.enter_context(tc.tile_pool(name="xs", bufs=1))

                # q/k projections (transposed form, bias per partition)
                with tc.tile_pool(name="pjqk", bufs=1, space="PSUM") as ppool:
                    for x_dram, w_sb, b_sb, dst in (
                        (qT, wq_sb, bq_sb, qhT),
                        (kT, wk_sb, bk_sb, khT),
                    ):
                        for nch2 in range(2):
                            psums = [ppool.tile([P, 1024], F32, tag=f"pj{p}", name=f"pj{p}", bufs=1)
                                     for p in range(NPAIR)]
                            for kt in range(KT):
                                xt = xpool.tile([P, 1024], MDT, tag="xt", bufs=4)
                                nc.sync.dma_start(
                                    out=xt,
                                    in_=x_dram.ap()[kt * P:(kt + 1) * P,
                                                    nch2 * 1024:(nch2 + 1) * 1024],
                                )
                                for p in range(NPAIR):
                                    for hf in range(2):
                                        nc.tensor.matmul(
                                            psums[p][:, hf * 512:(hf + 1) * 512],
                                            lhsT=_mm_ap(w_sb[:, kt, p * P:(p + 1) * P]),
                                            rhs=_mm_ap(xt[:, hf * 512:(hf + 1) * 512]),
                                            start=(kt == 0),
                                            stop=(kt == KT - 1),
                                        )
                            for p in range(NPAIR):
                                nc.scalar.activation(
                                    dst[:, p * S + nch2 * 1024: p * S + (nch2 + 1) * 1024],
                                    psums[p],
                                    AF.Relu,
                                    bias=b_sb[:, p: p + 1],
                                )

                # v projection (normal form, bias along free dim via DVE)
                with tc.tile_pool(name="pjv", bufs=1, space="PSUM") as vpool:
                    for stg in range(4):
                        pvs = [vpool.tile([P, 512], F32, tag=f"pv{i}", name=f"pv{i}", bufs=2)
                               for i in range(4)]
                        for kt in range(KT):
                            vt = xpool.tile([P, 512], MDT, tag="vt", bufs=4)
                            nc.sync.dma_start(
                                out=vt,
                                in_=vT.ap()[kt * P:(kt + 1) * P, stg * 512:(stg + 1) * 512],
                            )
                            for i in range(4):
                                nc.tensor.matmul(
                                    pvs[i],
                                    lhsT=_mm_ap(vt[:, i * P:(i + 1) * P]),
                                    rhs=_mm_ap(wv_sb[:, kt, :]),
                                    start=(kt == 0),
                                    stop=(kt == KT - 1),
                                )
                        for i in range(4):
                            st = stg * 4 + i
                            vslc = vh[:, st * DG:(st + 1) * DG]
                            nc.vector.tensor_tensor(vslc, pvs[i], bv_sb, ALU.add)
                            nc.vector.tensor_scalar(vslc, vslc, 0.0, None, ALU.max)

            # ---------- phase B: attention ----------
            with ExitStack() as b_ctx:
                apool = b_ctx.enter_context(tc.tile_pool(name="attn", bufs=3))
                epool = b_ctx.enter_context(tc.tile_pool(name="expt", bufs=3))
                rpool = b_ctx.enter_context(tc.tile_pool(name="recip", bufs=2))
                bpool = b_ctx.enter_context(tc.tile_pool(name="rbcast", bufs=2))
                ypool = b_ctx.enter_context(tc.tile_pool(name="ystage", bufs=2))
                sq_ps = b_ctx.enter_context(tc.tile_pool(name="sq", bufs=1, space="PSUM"))
                st_ps = b_ctx.enter_context(tc.tile_pool(name="st", bufs=1, space="PSUM"))
                yt_ps = b_ctx.enter_context(tc.tile_pool(name="yt", bufs=1, space="PSUM"))
                yt_bounce = t.get("yt_bounce")

                def emit_A(pr, hh, m, rec):
                    """q-major scores + softmax + attn output write for one q tile."""
                    h = 2 * pr + hh
                    qs = slice(64 * hh, 64 * hh + 64)
                    ps = sq_ps.tile([P, S], F32, tag="sq", name="sq")
                    for nch in range(4):
                        nc.tensor.matmul(
                            ps[:, nch * 512:(nch + 1) * 512],
                            lhsT=_mm_ap(qhT[qs, pr * S + m * P: pr * S + (m + 1) * P]),
                            rhs=_mm_ap(khT[qs, pr * S + nch * 512: pr * S + (nch + 1) * 512]),
                            start=True,
                            stop=True,
                        )
                    at = apool.tile([P, S], F32, tag="at", name="at")
                    den = rec[:, m: m + 1]
                    nc.scalar.activation(at, ps, AF.Exp, scale=SCALE, accum_out=den)
                    nc.vector.reciprocal(den, den)
                    nc.vector.tensor_scalar(at, at, den, None, ALU.mult)
                    nc.sync.dma_start(
                        out=attn_out.ap()[m * P:(m + 1) * P, h * S:(h + 1) * S],
                        in_=at,
                    )

                def emit_rb(h, rec):
                    """denominators: partition-form -> free-form broadcast tile."""
                    nc.gpsimd.dma_start(
                        out=recip_dram.ap()[h, :].rearrange("(m u) -> u m", u=P),
                        in_=rec,
                    )
                    rb = bpool.tile([P, S], F32, tag="rb", name="rb")
                    nc.gpsimd.dma_start(
                        out=rb,
                        in_=bass.AP(tensor=recip_dram, offset=h * S, ap=[[0, P], [1, S]]),
                    )
                    return rb

                def gen_B(pr, rbs):
                    """k-major scores + exp + attn@V; yields after each kt unit."""
                    for qh2 in range(2):
                        pyt = yt_ps.tile([P, S // 2], F32, tag="pyt", name="pyt")
                        for hh in range(2):
                            qs = slice(64 * hh, 64 * hh + 64)
                            for kt in range(QM):
                                pst = st_ps.tile([P, S // 2], F32, tag="pst", name="pst")
                                for nc2 in range(2):
                                    nc.tensor.matmul(
                                        pst[:, nc2 * 512:(nc2 + 1) * 512],
                                        lhsT=_mm_ap(khT[qs, pr * S + kt * P: pr * S + (kt + 1) * P]),
                                        rhs=_mm_ap(qhT[qs, pr * S + qh2 * 1024 + nc2 * 512:
                                                pr * S + qh2 * 1024 + (nc2 + 1) * 512]),
                                        start=True,
                                        stop=True,
                                    )
                                et = epool.tile([P, S // 2], mybir.dt.bfloat16, tag="et", name="et")
                                nc.scalar.activation(et, pst, AF.Exp, scale=SCALE)
                                h = 2 * pr + hh
                                for nc2 in range(2):
                                    nc.tensor.matmul(
                                        pyt[qs, nc2 * 512:(nc2 + 1) * 512],
                                        lhsT=vh[:, kt * DG + h * DH: kt * DG + (h + 1) * DH],
                                        rhs=et[:, nc2 * 512:(nc2 + 1) * 512],
                                        start=(kt == 0),
                                        stop=(kt == QM - 1),
                                    )
                                yield
                        # evict yT pair tile to DRAM, folding in normalization
                        ys = ypool.tile([P, S // 2], MDT, tag="ys", name="ys")
                        for hh in range(2):
                            qs = slice(64 * hh, 64 * hh + 64)
                            nc.vector.tensor_tensor(
                                ys[qs, :], pyt[qs, :],
                                rbs[hh][qs, qh2 * 1024:(qh2 + 1) * 1024],
                                ALU.mult,
                            )
                        if yt_bounce is not None:
                            nc.sync.dma_start(
                                out=yt_bounce.ap()[pr * P:(pr + 1) * P,
                                                   qh2 * 1024:(qh2 + 1) * 1024],
                                in_=ys,
                            )
                        yield

                ag_insts = []
                for pr in range(NPAIR):
                    # denominator tiles for both heads of the pair
                    recs = [rpool.tile([P, QM], F32, tag=f"rec{hh}", name=f"rec{hh}")
                            for hh in range(2)]
                    rbs = [None, None]

                    bgen = gen_B(pr - 1, _prev_rbs) if pr > 0 else iter(())
                    bdone = pr == 0
                    for hh in range(2):
                        for m in range(QM):
                            emit_A(pr, hh, m, recs[hh])
                            for _ in range(2):
                                if not bdone:
                                    try:
                                        next(bgen)
                                    except StopIteration:
                                        bdone = True
                        rbs[hh] = emit_rb(2 * pr + hh, recs[hh])
                    if not bdone:
                        for _ in bgen:
                            pass
                    if pr > 0 and USE_COLLECTIVE:
                        ag_insts.append(_emit_ag(nc, t, pr - 1))
                    _prev_rbs = rbs

                # drain the last pair's B stream
                for _ in gen_B(NPAIR - 1, _prev_rbs):
                    pass
                if USE_COLLECTIVE:
                    ag_insts.append(_emit_ag(nc, t, NPAIR - 1))

        # ---------- phase C: output projection ----------
        with ExitStack() as c_ctx:
            if USE_COLLECTIVE:
                opool = c_ctx.enter_context(tc.tile_pool(name="oproj", bufs=1))
                ytf = opool.tile([P, KT, S], MDT, tag="ytf")
                wo_sb = opool.tile([P, KT, D], MDT, tag="wo")
                # kk = j*4 + p (global dcol tile); load order p-major so the
                # last pair's chunks arrive last
                kk_order = [jj * NPAIR + pp for pp in range(NPAIR) for jj in range(2)]
                for kk in kk_order:
                    jj, pp = kk // NPAIR, kk % NPAIR
                    d2 = nc.sync.dma_start(
                        out=ytf[:, kk, :],
                        in_=t["yt_allp"][pp].ap()[jj, :, :],
                    )
                    tile.add_dep_helper(d2.ins, ag_insts[pp].ins, True, "ytf after AG")
                for kk in range(KT):
                    nc.sync.dma_start(
                        out=wo_sb[:, kk, :], in_=wo.ap()[kk * P:(kk + 1) * P, :]
                    )
                ypool = c_ctx.enter_context(tc.tile_pool(name="yev", bufs=2))
                o_ps = c_ctx.enter_context(tc.tile_pool(name="ops", bufs=4, space="PSUM"))
                for m in range(S // P):
                    po = o_ps.tile([P, D], F32, tag="po")
                    for ki, kk in enumerate(kk_order):
                        for nc2 in range(2):
                            nc.tensor.matmul(
                                po[:, nc2 * 512:(nc2 + 1) * 512],
                                lhsT=_mm_ap(ytf[:, kk, m * P:(m + 1) * P]),
                                rhs=_mm_ap(wo_sb[:, kk, nc2 * 512:(nc2 + 1) * 512]),
                                start=(ki == 0),
                                stop=(ki == KT - 1),
                            )
                    yo = ypool.tile([P, D], F32, tag="yo")
                    nc.vector.tensor_tensor(yo, po, bo_sb, ALU.add)
                    nc.vector.tensor_scalar(yo, yo, 0.0, None, ALU.max)
                    nc.sync.dma_start(out=y_out.ap()[m * P:(m + 1) * P, :], in_=yo)


def _get_nc():
    if "nc" not in _CACHE:
        _CACHE["nc"] = _build_nc()
    return _CACHE["nc"]


def _in_maps(q, k, v, Wq, bq, Wk, bk, Wv, bv, Wo, bo):
    maps = []
    WoT = np.ascontiguousarray(Wo.T)
    for c in range(8):
        b, g = c // 2, c % 2
        gs = slice(g * DG, (g + 1) * DG)
        maps.append({
            "qT": np.ascontiguousarray(q[b].T),
            "kT": np.ascontiguousarray(k[b].T),
            "vT": np.ascontiguousarray(v[b].T),
            "wq": np.ascontiguousarray(Wq[gs, :].T),
            "wk": np.ascontiguousarray(Wk[gs, :].T),
            "wv": np.ascontiguousarray(Wv[gs, :].T),
            "wo": WoT,
            "bq": np.ascontiguousarray(bq[gs].reshape(NPAIR, P).T),
            "bk": np.ascontiguousarray(bk[gs].reshape(NPAIR, P).T),
            "bv": np.ascontiguousarray(bv[gs].reshape(1, DG)),
            "bo": np.ascontiguousarray(bo.reshape(1, D)),
        })
    return maps


def run_spmd(inputs, **kwargs):
    """Run the SPMD kernel; returns (results_per_core, BassKernelResults)."""
    nc = _get_nc()
    in_maps = _in_maps(**inputs)
    res = run_bass_kernel_spmd(nc, in_maps, core_ids=list(range(8)), **kwargs)
    return res.results, res


def assemble(results):
    y = np.empty((B, S, D), dtype=np.float32)
    attn = np.empty((B, S, H * S), dtype=np.float32)
    for c in range(8):
        b, g = c // 2, c % 2
        attn[b, :, g * HL * S:(g + 1) * HL * S] = results[c]["attn_out"]
        half = slice(g * (S // 2), (g + 1) * (S // 2))
        y[b, half, :] = results[c]["y_out"][half, :]
    return y, attn


def kernel(**inputs):
    results, _ = run_spmd(inputs)
    return assemble(results)


# revision 27
# speedup vs baseline: 1.1151x; 1.0805x over previous
"""Trainium2 Bass kernel for nn_MultiHeadAttention (B=4, S=2048, D=1024, H=16).

Sharding over 8 NeuronCores: core c -> (batch b = c//2, head-group g = c%2).
Each core computes, for its batch and its 8 heads:
  - projections in transposed layout: qhT/khT [512, 2048] = relu(W x^T + b),
    vh in normal layout [2048, 512]
  - per head: scores twice (q-major for softmax/output write, k-major for
    attn@V), exp on ScalarE with fused row-sum (softmax denominator),
    normalization on VectorE, unnormalized attn@V on PE with the
    normalization folded into the PSUM eviction
  - output projection row-split across the 2 cores of a batch after a
    pairwise AllToAll exchange of y^T halves.
Outputs per core: its [2048, 16384] slab of attn_weights and its
[1024, 1024] slab of y.
"""

import os
import sys

for _p in ("/opt/trn_rl_repo", "/opt/pypackages"):
    if _p not in sys.path and os.path.isdir(_p):
        sys.path.append(_p)

import numpy as np

import concourse.bass as bass
import concourse.tile as tile
from concourse import bacc
from concourse import mybir
from concourse.bass_utils import run_bass_kernel_spmd

F32 = mybir.dt.float32
AF = mybir.ActivationFunctionType
ALU = mybir.AluOpType

P = 128          # partitions
B = 4            # batch
S = 2048         # sequence length
D = 1024         # model dim
H = 16           # total heads
DH = 64          # head dim
HL = 8           # heads per core (head-group size)
DG = HL * DH     # 512, projected dims per core
NPAIR = HL // 2  # 4 local head pairs
KT = D // P      # 8 contraction tiles for projections
QM = S // P      # 16 q-row tiles
SCALE = 1.0 / 8.0  # 1/sqrt(DH)

USE_COLLECTIVE = True
MM_DT = os.environ.get("MM_DT", "f32r")  # f32r | f32


_CACHE = {}


MDT = mybir.dt.float32r if os.environ.get("MM_DT", "f32r") == "f32r" else mybir.dt.float32


def _mm_ap(ap):
    return ap


def _build_nc():
    nc = bacc.Bacc("TRN2", target_bir_lowering=False, debug=False, num_devices=8)

    # ---- dram I/O ----
    qT = nc.dram_tensor("qT", [D, S], MDT, kind="ExternalInput")
    kT = nc.dram_tensor("kT", [D, S], MDT, kind="ExternalInput")
    vT = nc.dram_tensor("vT", [D, S], MDT, kind="ExternalInput")
    wq = nc.dram_tensor("wq", [D, DG], MDT, kind="ExternalInput")
    wk = nc.dram_tensor("wk", [D, DG], MDT, kind="ExternalInput")
    wv = nc.dram_tensor("wv", [D, DG], MDT, kind="ExternalInput")
    wo = nc.dram_tensor("wo", [D, D], MDT, kind="ExternalInput")
    bq = nc.dram_tensor("bq", [P, NPAIR], F32, kind="ExternalInput")
    bk = nc.dram_tensor("bk", [P, NPAIR], F32, kind="ExternalInput")
    bv = nc.dram_tensor("bv", [1, DG], F32, kind="ExternalInput")
    bo = nc.dram_tensor("bo", [1, D], F32, kind="ExternalInput")

    attn_out = nc.dram_tensor("attn_out", [S, HL * S], F32, kind="ExternalOutput")
    y_out = nc.dram_tensor("y_out", [S, D], F32, kind="ExternalOutput")

    # per-head softmax denominator bounce (for partition->free transposition)
    recip_dram = nc.dram_tensor("recip_dram", [HL, S], F32)
    if USE_COLLECTIVE:
        # pairwise AllGather: both cores of a batch receive [yT_g0; yT_g1],
        # split into 4 per-pair chunks so the exchange overlaps attention
        yt_bounce = nc.dram_tensor("yt_bounce", [DG, S], MDT)
        yt_allp = [nc.dram_tensor(f"yt_all{p}", [2, P, S], MDT) for p in range(NPAIR)]

    with tile.TileContext(nc) as tc:
        _emit(nc, tc, locals())
    if not nc.is_finalized():
        nc.finalize()
    return nc


def _emit_ag(nc, t, pr):
    yt_bounce = t["yt_bounce"]
    ag = nc.gpsimd.collective_compute(
        "AllGather",
        ALU.bypass,
        replica_groups=[[0, 1], [2, 3], [4, 5], [6, 7]],
        ins=[yt_bounce.ap()[pr * P:(pr + 1) * P, :].opt()],
        outs=[t["yt_allp"][pr].ap().opt()],
    )
    return ag


def _emit(nc, tc, t):
    from contextlib import ExitStack

    qT, kT, vT = t["qT"], t["kT"], t["vT"]
    wq, wk, wv, wo = t["wq"], t["wk"], t["wv"], t["wo"]
    bq, bk, bv, bo = t["bq"], t["bk"], t["bv"], t["bo"]
    attn_out, y_out, recip_dram = t["attn_out"], t["y_out"], t["recip_dram"]

    with ExitStack() as ctx:
        # ---------- constants ----------
        const = ctx.enter_context(tc.tile_pool(name="const", bufs=1))
        bq_sb = const.tile([P, NPAIR], F32, tag="bq")
        bk_sb = const.tile([P, NPAIR], F32, tag="bk")
        bv_sb = const.tile([P, DG], F32, tag="bv")
        bo_sb = const.tile([P, D], F32, tag="bo")
        nc.sync.dma_start(out=bq_sb, in_=bq.ap())
        nc.sync.dma_start(out=bk_sb, in_=bk.ap())
        nc.sync.dma_start(out=bv_sb, in_=bass.AP(tensor=bv, offset=0, ap=[[0, P], [1, DG]]))
        nc.sync.dma_start(out=bo_sb, in_=bass.AP(tensor=bo, offset=0, ap=[[0, P], [1, D]]))

        # ---------- resident activations ----------
        # qhT/khT: [128, pair-block p * 2048 + s]; partitions 0:64 = head 2p,
        # 64:128 = head 2p+1 (feature dim on partitions).
        # vh: [128, s-tile st * 512 + d_local] (sequence on partitions).
        # qhT doubles as the y^T staging buffer late in the kernel.
        qy_pool = ctx.enter_context(tc.tile_pool(name="qy", bufs=1))
        qhT = qy_pool.tile([P, NPAIR * S], MDT, tag="qhT")

        with ExitStack() as attn_ctx:
            kv_pool = attn_ctx.enter_context(tc.tile_pool(name="kv", bufs=1))
            khT = kv_pool.tile([P, NPAIR * S], MDT, tag="khT")
            vh = kv_pool.tile([P, QM * DG], mybir.dt.bfloat16, tag="vh")

            # ---------- phase A: projections ----------
            with ExitStack() as proj_ctx:
                wpool = proj_ctx.enter_context(tc.tile_pool(name="wts", bufs=1))
                wq_sb = wpool.tile([P, KT, DG], MDT, tag="wq")
                wk_sb = wpool.tile([P, KT, DG], MDT, tag="wk")
                wv_sb = wpool.tile([P, KT, DG], MDT, tag="wv")
                for w_dram, w_sb in ((wq, wq_sb), (wk, wk_sb), (wv, wv_sb)):
                    for kt in range(KT):
                        nc.sync.dma_start(
                            out=w_sb[:, kt, :],
                            in_=w_dram.ap()[kt * P:(kt + 1) * P, :],
                        )

                xpool = proj_ctx.enter_context(tc.tile_pool(name="xs", bufs=1))

                # q/k projections (transposed form, bias per partition)
                with tc.tile_pool(name="pjqk", bufs=1, space="PSUM") as ppool:
                    for x_dram, w_sb, b_sb, dst in (
                        (qT, wq_sb, bq_sb, qhT),
                        (kT, wk_sb, bk_sb, khT),
                    ):
                        for nch2 in range(2):
                            psums = [ppool.tile([P, 1024], F32, tag=f"pj{p}", name=f"pj{p}", bufs=1)
                                     for p in range(NPAIR)]
                            for kt in range(KT):
                                xt = xpool.tile([P, 1024], MDT, tag="xt", bufs=4)
                                nc.sync.dma_start(
                                    out=xt,
                                    in_=x_dram.ap()[kt * P:(kt + 1) * P,
                                                    nch2 * 1024:(nch2 + 1) * 1024],
                                )
                                for p in range(NPAIR):
                                    for hf in range(2):
                                        nc.tensor.matmul(
                                            psums[p][:, hf * 512:(hf + 1) * 512],
                                            lhsT=_mm_ap(w_sb[:, kt, p * P:(p + 1) * P]),
                                            rhs=_mm_ap(xt[:, hf * 512:(hf + 1) * 512]),
                                            start=(kt == 0),
                                            stop=(kt == KT - 1),
                                        )
                            for p in range(NPAIR):
                                nc.scalar.activation(
                                    dst[:, p * S + nch2 * 1024: p * S + (nch2 + 1) * 1024],
                                    psums[p],
                                    AF.Relu,
                                    bias=b_sb[:, p: p + 1],
                                )

                # v projection (normal form, bias along free dim via DVE)
                with tc.tile_pool(name="pjv", bufs=1, space="PSUM") as vpool:
                    for stg in range(4):
                        pvs = [vpool.tile([P, 512], F32, tag=f"pv{i}", name=f"pv{i}", bufs=2)
                               for i in range(4)]
                        for kt in range(KT):
                            vt = xpool.tile([P, 512], MDT, tag="vt", bufs=4)
                            nc.sync.dma_start(
                                out=vt,
                                in_=vT.ap()[kt * P:(kt + 1) * P, stg * 512:(stg + 1) * 512],
                            )
                            for i in range(4):
                                nc.tensor.matmul(
                                    pvs[i],
                                    lhsT=_mm_ap(vt[:, i * P:(i + 1) * P]),
                                    rhs=_mm_ap(wv_sb[:, kt, :]),
                                    start=(kt == 0),
                                    stop=(kt == KT - 1),
                                )
                        for i in range(4):
                            st = stg * 4 + i
                            vslc = vh[:, st * DG:(st + 1) * DG]
                            nc.vector.tensor_tensor(vslc, pvs[i], bv_sb, ALU.add)
                            nc.vector.tensor_scalar(vslc, vslc, 0.0, None, ALU.max)

            # ---------- phase B: attention ----------
            with ExitStack() as b_ctx:
                apool = b_ctx.enter_context(tc.tile_pool(name="attn", bufs=3))
                epool = b_ctx.enter_context(tc.tile_pool(name="expt", bufs=6))
                rpool = b_ctx.enter_context(tc.tile_pool(name="recip", bufs=2))
                bpool = b_ctx.enter_context(tc.tile_pool(name="rbcast", bufs=2))
                ypool = b_ctx.enter_context(tc.tile_pool(name="ystage", bufs=2))
                sq_ps = b_ctx.enter_context(tc.tile_pool(name="sq", bufs=1, space="PSUM"))
                st_ps = b_ctx.enter_context(tc.tile_pool(name="st", bufs=1, space="PSUM"))
                yt_ps = b_ctx.enter_context(tc.tile_pool(name="yt", bufs=1, space="PSUM"))
                yt_bounce = t.get("yt_bounce")

                def emit_A(pr, hh, m, rec):
                    """q-major scores + softmax + attn output write for one q tile."""
                    h = 2 * pr + hh
                    qs = slice(64 * hh, 64 * hh + 64)
                    ps = sq_ps.tile([P, S], F32, tag="sq", name="sq")
                    for nch in range(4):
                        nc.tensor.matmul(
                            ps[:, nch * 512:(nch + 1) * 512],
                            lhsT=_mm_ap(qhT[qs, pr * S + m * P: pr * S + (m + 1) * P]),
                            rhs=_mm_ap(khT[qs, pr * S + nch * 512: pr * S + (nch + 1) * 512]),
                            start=True,
                            stop=True,
                        )
                    at = apool.tile([P, S], F32, tag="at", name="at")
                    den = rec[:, m: m + 1]
                    nc.scalar.activation(at, ps, AF.Exp, scale=SCALE, accum_out=den)
                    nc.vector.reciprocal(den, den)
                    nc.vector.tensor_scalar(at, at, den, None, ALU.mult)
                    nc.sync.dma_start(
                        out=attn_out.ap()[m * P:(m + 1) * P, h * S:(h + 1) * S],
                        in_=at,
                    )

                def emit_rb(h, rec):
                    """denominators: partition-form -> free-form broadcast tile."""
                    nc.gpsimd.dma_start(
                        out=recip_dram.ap()[h, :].rearrange("(m u) -> u m", u=P),
                        in_=rec,
                    )
                    rb = bpool.tile([P, S], F32, tag="rb", name="rb")
                    nc.gpsimd.dma_start(
                        out=rb,
                        in_=bass.AP(tensor=recip_dram, offset=h * S, ap=[[0, P], [1, S]]),
                    )
                    return rb

                def gen_B(pr, rbs):
                    """k-major scores + exp + attn@V, batched by 4 kt to limit
                    PE tile-mode switches; yields after each batch."""
                    KB = 4
                    for qh2 in range(2):
                        pyt = yt_ps.tile([P, S // 2], F32, tag="pyt", name="pyt")
                        for hh in range(2):
                            qs = slice(64 * hh, 64 * hh + 64)
                            h = 2 * pr + hh
                            for kb in range(QM // KB):
                                ets = []
                                for kt in range(kb * KB, (kb + 1) * KB):
                                    pst = st_ps.tile([P, S // 2], F32, tag="pst", name="pst")
                                    for nc2 in range(2):
                                        nc.tensor.matmul(
                                            pst[:, nc2 * 512:(nc2 + 1) * 512],
                                            lhsT=_mm_ap(khT[qs, pr * S + kt * P: pr * S + (kt + 1) * P]),
                                            rhs=_mm_ap(qhT[qs, pr * S + qh2 * 1024 + nc2 * 512:
                                                    pr * S + qh2 * 1024 + (nc2 + 1) * 512]),
                                            start=True,
                                            stop=True,
                                        )
                                    et = epool.tile([P, S // 2], mybir.dt.bfloat16, tag="et", name="et")
                                    nc.scalar.activation(et, pst, AF.Exp, scale=SCALE)
                                    ets.append((kt, et))
                                for kt, et in ets:
                                    for nc2 in range(2):
                                        nc.tensor.matmul(
                                            pyt[qs, nc2 * 512:(nc2 + 1) * 512],
                                            lhsT=vh[:, kt * DG + h * DH: kt * DG + (h + 1) * DH],
                                            rhs=et[:, nc2 * 512:(nc2 + 1) * 512],
                                            start=(kt == 0),
                                            stop=(kt == QM - 1),
                                        )
                                yield
                        # evict yT pair tile to DRAM, folding in normalization
                        ys = ypool.tile([P, S // 2], MDT, tag="ys", name="ys")
                        for hh in range(2):
                            qs = slice(64 * hh, 64 * hh + 64)
                            nc.vector.tensor_tensor(
                                ys[qs, :], pyt[qs, :],
                                rbs[hh][qs, qh2 * 1024:(qh2 + 1) * 1024],
                                ALU.mult,
                            )
                        if yt_bounce is not None:
                            nc.sync.dma_start(
                                out=yt_bounce.ap()[pr * P:(pr + 1) * P,
                                                   qh2 * 1024:(qh2 + 1) * 1024],
                                in_=ys,
                            )
                        yield

                ag_insts = []
                for pr in range(NPAIR):
                    # denominator tiles for both heads of the pair
                    recs = [rpool.tile([P, QM], F32, tag=f"rec{hh}", name=f"rec{hh}")
                            for hh in range(2)]
                    rbs = [None, None]

                    bgen = gen_B(pr - 1, _prev_rbs) if pr > 0 else iter(())
                    bdone = pr == 0
                    for hh in range(2):
                        for m in range(QM):
                            emit_A(pr, hh, m, recs[hh])
                            if m % 2 == 0 and not bdone:
                                try:
                                    next(bgen)
                                except StopIteration:
                                    bdone = True
                        rbs[hh] = emit_rb(2 * pr + hh, recs[hh])
                    if not bdone:
                        for _ in bgen:
                            pass
                    if pr > 0 and USE_COLLECTIVE:
                        ag_insts.append(_emit_ag(nc, t, pr - 1))
                    _prev_rbs = rbs

                # drain the last pair's B stream
                for _ in gen_B(NPAIR - 1, _prev_rbs):
                    pass
                if USE_COLLECTIVE:
                    ag_insts.append(_emit_ag(nc, t, NPAIR - 1))

        # ---------- phase C: output projection ----------
        with ExitStack() as c_ctx:
            if USE_COLLECTIVE:
                opool = c_ctx.enter_context(tc.tile_pool(name="oproj", bufs=1))
                ytf = opool.tile([P, KT, S], MDT, tag="ytf")
                wo_sb = opool.tile([P, KT, D], MDT, tag="wo")
                # kk = j*4 + p (global dcol tile); load order p-major so the
                # last pair's chunks arrive last
                kk_order = [jj * NPAIR + pp for pp in range(NPAIR) for jj in range(2)]
                for kk in kk_order:
                    jj, pp = kk // NPAIR, kk % NPAIR
                    d2 = nc.sync.dma_start(
                        out=ytf[:, kk, :],
                        in_=t["yt_allp"][pp].ap()[jj, :, :],
                    )
                    tile.add_dep_helper(d2.ins, ag_insts[pp].ins, True, "ytf after AG")
                for kk in range(KT):
                    nc.sync.dma_start(
                        out=wo_sb[:, kk, :], in_=wo.ap()[kk * P:(kk + 1) * P, :]
                    )
                ypool = c_ctx.enter_context(tc.tile_pool(name="yev", bufs=2))
                o_ps = c_ctx.enter_context(tc.tile_pool(name="ops", bufs=4, space="PSUM"))
                for m in range(S // P):
                    po = o_ps.tile([P, D], F32, tag="po")
                    for ki, kk in enumerate(kk_order):
                        for nc2 in range(2):
                            nc.tensor.matmul(
                                po[:, nc2 * 512:(nc2 + 1) * 512],
                                lhsT=_mm_ap(ytf[:, kk, m * P:(m + 1) * P]),
                                rhs=_mm_ap(wo_sb[:, kk, nc2 * 512:(nc2 + 1) * 512]),
                                start=(ki == 0),
                                stop=(ki == KT - 1),
                            )
                    yo = ypool.tile([P, D], F32, tag="yo")
                    nc.vector.tensor_tensor(yo, po, bo_sb, ALU.add)
                    nc.vector.tensor_scalar(yo, yo, 0.0, None, ALU.max)
                    nc.sync.dma_start(out=y_out.ap()[m * P:(m + 1) * P, :], in_=yo)


def _get_nc():
    if "nc" not in _CACHE:
        _CACHE["nc"] = _build_nc()
    return _CACHE["nc"]


def _in_maps(q, k, v, Wq, bq, Wk, bk, Wv, bv, Wo, bo):
    maps = []
    WoT = np.ascontiguousarray(Wo.T)
    for c in range(8):
        b, g = c // 2, c % 2
        gs = slice(g * DG, (g + 1) * DG)
        maps.append({
            "qT": np.ascontiguousarray(q[b].T),
            "kT": np.ascontiguousarray(k[b].T),
            "vT": np.ascontiguousarray(v[b].T),
            "wq": np.ascontiguousarray(Wq[gs, :].T),
            "wk": np.ascontiguousarray(Wk[gs, :].T),
            "wv": np.ascontiguousarray(Wv[gs, :].T),
            "wo": WoT,
            "bq": np.ascontiguousarray(bq[gs].reshape(NPAIR, P).T),
            "bk": np.ascontiguousarray(bk[gs].reshape(NPAIR, P).T),
            "bv": np.ascontiguousarray(bv[gs].reshape(1, DG)),
            "bo": np.ascontiguousarray(bo.reshape(1, D)),
        })
    return maps


def run_spmd(inputs, **kwargs):
    """Run the SPMD kernel; returns (results_per_core, BassKernelResults)."""
    nc = _get_nc()
    in_maps = _in_maps(**inputs)
    res = run_bass_kernel_spmd(nc, in_maps, core_ids=list(range(8)), **kwargs)
    return res.results, res


def assemble(results):
    y = np.empty((B, S, D), dtype=np.float32)
    attn = np.empty((B, S, H * S), dtype=np.float32)
    for c in range(8):
        b, g = c // 2, c % 2
        attn[b, :, g * HL * S:(g + 1) * HL * S] = results[c]["attn_out"]
        half = slice(g * (S // 2), (g + 1) * (S // 2))
        y[b, half, :] = results[c]["y_out"][half, :]
    return y, attn


def kernel(**inputs):
    results, _ = run_spmd(inputs)
    return assemble(results)
